# revision 14
# baseline (speedup 1.0000x reference)
"""ConvShiftLayer TRN2 kernel: a = tanh(x@W+b); z = (a>0); z_conv = shift-conv(z).

Math: z_conv[t, o] = sum_{d=0..7} z[t+4-d, (o-d) % 1024]  (zero outside seq range)
Factored: A[p, v] = sum_{d0=0..3} z[v-d0, (p-d0)%F]; C[o, u] = A[o, u+7] + A[(o-4)%F, u+3].

Sharding: 8 cores = (batch 4) x (seq halves 2); each core computes a 512-step seq
slice + halo (3 left / 4 right) from scratch. On-chip layout: features on
partitions (8 tiles of 128), seq on the free axis (520 cols: 3 halo + 512 owned
+ 4 halo + 1 pad).

Main matmul: 3-term bf16 split (xh@wh + xl@wh + xh@wl) accumulated in fp32 psum
(error ~2^-16, exact enough for the z threshold). Bias enters via a 2-row
(bh, bl) weight block against a masked ones vector, so halo/pad columns get
pre-activation exactly 0 -> z = 0 (SAME-conv zero padding), with full bias
precision on valid columns.

Conv: banded shift-matrix matmuls in bf16 (exact on 0/1 data), accumulated in
psum. Stage A applies (I + Q + Q^2 + Q^3), stage B (I + Q^4), where Q's feature
shift is the lhsT matrix (in-tile band + cross-tile wrap) and its time shift is
a free-axis column offset of the rhs. No DRAM round trips.

Outputs stream back as bf16 (a: ~0.2% rounding, ok at 2e-2 tol; z and z_conv are
small ints, exact); the host converts to f32.
"""
import numpy as np
import ml_dtypes
from contextlib import ExitStack

import concourse.bass as bass
import concourse.mybir as mybir
from concourse.bass_utils import run_bass_kernel_spmd

F_DIM = 1024
IN_DIM = 768
SEQ = 1024
BATCH = 4
T = 520          # 3 halo + 512 owned + 4 halo + 1 pad
OWN = 512
NK = 6           # K tiles of 128 over 768
NF = 8           # feature tiles of 128

f32 = mybir.dt.float32
bf16 = mybir.dt.bfloat16
bf16_np = ml_dtypes.bfloat16

A_ORDER = [0, 1, 2, 3, 4, 5, 6, 7]   # stage A tile order (A_f needs z_f, z_{f+1})
B_ORDER = [0, 1, 2, 3, 4, 5, 6, 7]   # stage B tile order (B_i needs A_i, A_{i+1})

LAST_RESULTS = None  # BassKernelResults of the most recent run (for test.py)


def _build_module():
    nc = bass.Bass()
    xh_in = nc.declare_dram_parameter("xh", [NK * 128, T], bf16, isOutput=False)
    xl_in = nc.declare_dram_parameter("xl", [NK * 128, T], bf16, isOutput=False)
    ones_in = nc.declare_dram_parameter("ones2", [2, T], bf16, isOutput=False)
    wh_in = nc.declare_dram_parameter("wh", [NK * NF * 128, 128], bf16, isOutput=False)
    wl_in = nc.declare_dram_parameter("wl", [NK * NF * 128, 128], bf16, isOutput=False)
    bias_in = nc.declare_dram_parameter("bias2", [2, F_DIM], bf16, isOutput=False)
    sh_in = nc.declare_dram_parameter("sh", [128, 9 * 128], bf16, isOutput=False)
    at_out = nc.declare_dram_parameter("at", [F_DIM, OWN], bf16, isOutput=True)
    zt_out = nc.declare_dram_parameter("zt", [F_DIM, OWN], bf16, isOutput=True)
    ct_out = nc.declare_dram_parameter("ct", [F_DIM, OWN], bf16, isOutput=True)

    ctx = ExitStack()
    with ctx:
        xh = [ctx.enter_context(nc.sbuf_tensor(f"xh{k}", [128, T], bf16)) for k in range(NK)]
        xl = [ctx.enter_context(nc.sbuf_tensor(f"xl{k}", [128, T], bf16)) for k in range(NK)]
        ones2 = ctx.enter_context(nc.sbuf_tensor("ones2s", [2, T], bf16))
        wh = [ctx.enter_context(nc.sbuf_tensor(f"wh{k}", [128, F_DIM], bf16)) for k in range(NK)]
        wl = [ctx.enter_context(nc.sbuf_tensor(f"wl{k}", [128, F_DIM], bf16)) for k in range(NK)]
        bias2 = ctx.enter_context(nc.sbuf_tensor("bias2s", [2, F_DIM], bf16))
        sh = ctx.enter_context(nc.sbuf_tensor("shs", [128, 9 * 128], bf16))
        a_s = [ctx.enter_context(nc.sbuf_tensor(f"a{f}", [128, T], bf16)) for f in range(NF)]
        z_s = [ctx.enter_context(nc.sbuf_tensor(f"z{f}", [128, T], bf16)) for f in range(NF)]
        A_s = [ctx.enter_context(nc.sbuf_tensor(f"A{f}", [128, 516], bf16)) for f in range(NF)]
        cc = [ctx.enter_context(nc.sbuf_tensor(f"cc{f}", [128, OWN], bf16)) for f in range(NF)]
        # psum: 4 banks main (pingpong pairs of 260+260), 4 banks stage A
        # (pingpong pairs of 258+258); stage B reuses the main banks.
        mA = [ctx.enter_context(nc.psum_tensor(f"mA{i}", [128, 260], f32)) for i in range(2)]
        mB = [ctx.enter_context(nc.psum_tensor(f"mB{i}", [128, 260], f32)) for i in range(2)]
        pA = [ctx.enter_context(nc.psum_tensor(f"pA{i}", [128, 258], f32)) for i in range(4)]

        dxh = ctx.enter_context(nc.semaphore("dxh"))
        dwh = ctx.enter_context(nc.semaphore("dwh"))
        dxl = ctx.enter_context(nc.semaphore("dxl"))
        dwl = ctx.enter_context(nc.semaphore("dwl"))
        smm = ctx.enter_context(nc.semaphore("smm"))
        sza = ctx.enter_context(nc.semaphore("sza"))
        szz = ctx.enter_context(nc.semaphore("szz"))
        sAmm = ctx.enter_context(nc.semaphore("sAmm"))
        sAcp = ctx.enter_context(nc.semaphore("sAcp"))
        sBmm = ctx.enter_context(nc.semaphore("sBmm"))
        sCcp = ctx.enter_context(nc.semaphore("sCcp"))
        dout = ctx.enter_context(nc.semaphore("dout"))

        block = ctx.enter_context(nc.Block())

        # shift matrices in sh: slot d (0..4) = S_d  (S_d[p, o] = 1 iff p == o+d),
        # slot 4+d (d=1..4) = W_d (W_d[p, o] = 1 iff p == o+d-128)
        def S(d):
            return sh[:, 128 * d:128 * (d + 1)]

        def Wr(d):
            return sh[:, 128 * (4 + d):128 * (5 + d)]

        @block.sync
        def _(sync):
            # inputs, in consumption order. Per-f gating thresholds require
            # ordered completion; DMA queues don't order across streams, so
            # the sync queue serializes weight-block groups with waits.
            for k in range(NK):
                sync.dma_start(out=xh[k][:, :], in_=xh_in[128 * k:128 * (k + 1), :]).then_inc(dxh, 16)
            sync.dma_start(out=ones2[:, :], in_=ones_in[:, :]).then_inc(dxh, 16)
            sync.dma_start(out=sh[:, :], in_=sh_in[:, :]).then_inc(dxh, 16)
            for k in range(NK):
                sync.dma_start(out=xl[k][:, :], in_=xl_in[128 * k:128 * (k + 1), :]).then_inc(dxl, 16)
            for k in range(NK):  # wh f0 blocks
                r = (k * NF + 0) * 128
                sync.dma_start(out=wh[k][:, 0:128], in_=wh_in[r:r + 128, :]).then_inc(dwh, 16)
            sync.dma_start(out=bias2[:, :], in_=bias_in[:, :]).then_inc(dwh, 16)
            for k in range(NK):
                r = (k * NF + 0) * 128
                sync.dma_start(out=wl[k][:, 0:128], in_=wl_in[r:r + 128, :]).then_inc(dwl, 16)
            for f in range(1, NF):
                sync.wait_ge(dwh, 16 * (7 + NK * (f - 1)))   # wh group f-1 (+bias) done
                for k in range(NK):
                    r = (k * NF + f) * 128
                    sync.dma_start(out=wh[k][:, 128 * f:128 * (f + 1)],
                                   in_=wh_in[r:r + 128, :]).then_inc(dwh, 16)
                sync.wait_ge(dwl, 16 * NK * f)               # wl group f-1 done
                for k in range(NK):
                    r = (k * NF + f) * 128
                    sync.dma_start(out=wl[k][:, 128 * f:128 * (f + 1)],
                                   in_=wl_in[r:r + 128, :]).then_inc(dwl, 16)
            # outputs as they become ready
            for f in range(NF):
                sync.wait_ge(sza, 2 * (f + 1))
                sync.dma_start(out=at_out[128 * f:128 * (f + 1), :], in_=a_s[f][:, 3:515]).then_inc(dout, 16)
                sync.wait_ge(szz, f + 1)
                sync.dma_start(out=zt_out[128 * f:128 * (f + 1), :], in_=z_s[f][:, 3:515]).then_inc(dout, 16)
            for pos, i in enumerate(B_ORDER):
                sync.wait_ge(sCcp, 2 * (pos + 1))
                sync.dma_start(out=ct_out[128 * i:128 * (i + 1), :], in_=cc[i][:, :]).then_inc(dout, 16)
            sync.wait_ge(dout, 16 * 3 * NF)

        @block.tensor
        def _(tensor):
            def stage_a(idx, i):
                # A_i[o, v] = sum_{d=0..3} z[(128i + o + d) % 1024, v - d], v in [3,519)
                if idx == 0:
                    tensor.wait_ge(dxh, 16 * 8)      # shift matrices
                if idx >= 2:
                    tensor.wait_ge(sAcp, 2 * (idx - 1))   # pingpong slot free
                tensor.wait_ge(szz, min(i + 2, NF))
                for half in range(2):
                    c0 = 3 + 258 * half
                    p = pA[2 * (idx % 2) + half]
                    for d in range(4):
                        ins = tensor.matmul(p[:, :], lhsT=S(d),
                                            rhs=z_s[i][:, c0 - d:c0 - d + 258],
                                            start=(d == 0), stop=False)
                    for d in range(1, 4):
                        ins = tensor.matmul(p[:, :], lhsT=Wr(d),
                                            rhs=z_s[(i + 1) % NF][:, c0 - d:c0 - d + 258],
                                            start=False, stop=(d == 3))
                    ins.then_inc(sAmm, 1)

            for f in range(NF):
                # gates: all xh/ones, all xl, wh f-blocks + bias, wl f-blocks
                if f == 0:
                    tensor.wait_ge(dxh, 16 * 8)
                    tensor.wait_ge(dxl, 16 * 6)
                tensor.wait_ge(dwh, 16 * (7 + NK * f))
                tensor.wait_ge(dwl, 16 * NK * (f + 1))
                if f >= 2:
                    tensor.wait_ge(sza, 2 * (f - 1))  # pingpong banks free
                fc = slice(128 * f, 128 * (f + 1))
                for half, bank in ((0, mA[f % 2]), (1, mB[f % 2])):
                    c0 = 260 * half
                    cs = slice(c0, c0 + 260)
                    for k in range(NK):
                        tensor.matmul(bank[:, :], lhsT=wh[k][:, fc], rhs=xh[k][:, cs],
                                      start=(k == 0), stop=False)
                    tensor.matmul(bank[:, :], lhsT=bias2[:, fc], rhs=ones2[:, cs],
                                  start=False, stop=False)
                    for k in range(NK):
                        tensor.matmul(bank[:, :], lhsT=wh[k][:, fc], rhs=xl[k][:, cs],
                                      start=False, stop=False)
                    for k in range(NK):
                        ins = tensor.matmul(bank[:, :], lhsT=wl[k][:, fc], rhs=xh[k][:, cs],
                                            start=False, stop=(k == NK - 1))
                    ins.then_inc(smm, 1)
                # interleave stage A behind the main matmuls
                if f >= 1:
                    stage_a(f - 1, A_ORDER[f - 1])
            stage_a(7, A_ORDER[7])

            tensor.wait_ge(sza, 2 * NF)  # main psum banks all drained
            for pos, i in enumerate(B_ORDER):
                # C_i[o, u] = A[128i+o, u+7] + A[(128i+o+4)%1024, u+3], u in [0,512)
                # A_s[i][:, j] holds A[128i: , j+3]
                idxA = A_ORDER.index(i)
                idxA1 = A_ORDER.index((i + 1) % NF)
                tensor.wait_ge(sAcp, 2 * (max(idxA, idxA1) + 1))
                if pos >= 2:
                    tensor.wait_ge(sCcp, 2 * (pos - 1))  # pingpong banks free
                for half, bank in ((0, mA[pos % 2]), (1, mB[pos % 2])):
                    c0 = 256 * half
                    ins = tensor.matmul(bank[:, 0:256], lhsT=S(0),
                                        rhs=A_s[i][:, c0 + 4:c0 + 260], start=True, stop=False)
                    ins = tensor.matmul(bank[:, 0:256], lhsT=S(4),
                                        rhs=A_s[i][:, c0:c0 + 256], start=False, stop=False)
                    ins = tensor.matmul(bank[:, 0:256], lhsT=Wr(4),
                                        rhs=A_s[(i + 1) % NF][:, c0:c0 + 256], start=False, stop=True)
                    ins.then_inc(sBmm, 1)

        @block.scalar
        def _(scalar):
            for f in range(NF):
                for half, bank in ((0, mA[f % 2]), (1, mB[f % 2])):
                    scalar.wait_ge(smm, 2 * f + half + 1)
                    scalar.activation(out=a_s[f][:, 260 * half:260 * (half + 1)], in_=bank[:, :],
                                      func=mybir.ActivationFunctionType.Tanh).then_inc(sza, 1)
            for pos, i in enumerate(B_ORDER):
                for half, bank in ((0, mA[pos % 2]), (1, mB[pos % 2])):
                    scalar.wait_ge(sBmm, 2 * pos + half + 1)
                    scalar.activation(out=cc[i][:, 256 * half:256 * (half + 1)], in_=bank[:, 0:256],
                                      func=mybir.ActivationFunctionType.Copy).then_inc(sCcp, 1)

        @block.vector
        def _(vector):
            def copy_a(idx):
                i = A_ORDER[idx]
                for half in range(2):
                    vector.wait_ge(sAmm, 2 * idx + half + 1)
                    vector.tensor_scalar_add(out=A_s[i][:, 258 * half:258 * (half + 1)],
                                             in0=pA[2 * (idx % 2) + half][:, :],
                                             scalar1=0.0).then_inc(sAcp, 1)

            # interleave to match the tensor stream: stage_a(idx) is issued
            # after M_{idx+1}, so copy(idx) follows is_gt_{idx+1}
            for f in range(NF):
                vector.wait_ge(sza, 2 * (f + 1))
                vector.tensor_scalar(out=z_s[f][:, :], in0=a_s[f][:, :], scalar1=0.0,
                                     scalar2=None, op0=mybir.AluOpType.is_gt).then_inc(szz, 1)
                if f >= 1:
                    copy_a(f - 1)
            copy_a(7)

    return nc


_NC = None


def _prep_weights(W, b):
    wh4 = W.astype(bf16_np)
    wl4 = (W - wh4.astype(np.float32)).astype(bf16_np)
    # pack (k, f) blocks contiguously: [6, 128, 8, 128] -> [(k f), 128rows, 128cols]
    def pack(w):
        return np.ascontiguousarray(
            w.reshape(NK, 128, NF, 128).transpose(0, 2, 1, 3).reshape(NK * NF * 128, 128))
    bh = b.astype(bf16_np)
    bl = (b - bh.astype(np.float32)).astype(bf16_np)
    bias2 = np.stack([bh, bl], axis=0)  # [2, 1024]

    # S_d[p, o] = 1 iff p == o + d; W_d[p, o] = 1 iff p == o + d - 128
    s = np.zeros((128, 9 * 128), dtype=bf16_np)
    for d in range(5):
        for o in range(0, 128 - d):
            s[o + d, 128 * d + o] = 1
    for d in range(1, 5):
        for o in range(128 - d, 128):
            s[o + d - 128, 128 * (4 + d) + o] = 1
    return pack(wh4), pack(wl4), bias2, s


def kernel(x: np.ndarray, W: np.ndarray, b: np.ndarray):
    global _NC, LAST_RESULTS
    x = np.asarray(x, dtype=np.float32)
    W = np.asarray(W, dtype=np.float32)
    b = np.asarray(b, dtype=np.float32)

    if _NC is None:
        _NC = _build_module()
    nc = _NC

    wh_p, wl_p, bias2, shifts = _prep_weights(W, b)

    in_maps = []
    metas = []
    for c in range(8):
        bi, h = c // 2, c % 2
        t0 = OWN * h
        lo, hi = t0 - 3, t0 + 517  # 520 rows
        xc = np.zeros((T, IN_DIM), dtype=np.float32)
        ones = np.zeros((T,), dtype=np.float32)
        src_lo, src_hi = max(lo, 0), min(hi, SEQ)
        xc[src_lo - lo:src_hi - lo, :] = x[bi, src_lo:src_hi, :]
        ones[src_lo - lo:src_hi - lo] = 1.0
        ones[T - 1] = 0.0  # pad col
        xt = np.ascontiguousarray(xc.T)              # [768, 520] f32
        xh = xt.astype(bf16_np)
        xlr = (xt - xh.astype(np.float32)).astype(bf16_np)
        ones2 = np.broadcast_to(ones.astype(bf16_np), (2, T)).copy()
        in_maps.append({"xh": xh, "xl": xlr, "ones2": ones2, "wh": wh_p, "wl": wl_p,
                        "bias2": bias2, "sh": shifts})
        metas.append((bi, t0))

    res = run_bass_kernel_spmd(nc, in_maps, list(range(8)))
    LAST_RESULTS = res

    a_full = np.empty((BATCH, SEQ, F_DIM), dtype=np.float32)
    z_full = np.empty((BATCH, SEQ, F_DIM), dtype=np.float32)
    zc_full = np.empty((BATCH, SEQ, F_DIM), dtype=np.float32)
    for c in range(8):
        bi, t0 = metas[c]
        r = res.results[c]
        a_full[bi, t0:t0 + OWN, :] = r["at"].astype(np.float32).T
        z_full[bi, t0:t0 + OWN, :] = r["zt"].astype(np.float32).T
        zc_full[bi, t0:t0 + OWN, :] = r["ct"].astype(np.float32).T
    return (a_full, z_full, zc_full)
